# revision 18
# baseline (speedup 1.0000x reference)
"""Slot-attention kernel for Trainium2, SPMD over 8 NeuronCores.

Math (per batch b):
    s = keys @ query.T / sqrt(64)            # (N, 8)
    e = exp(s)                               # softmax over slots, no max-sub
    p = e / rowsum(e)                        # (N, 8)
    out = (p.T @ values) / (p.T @ ones)      # (8, 64)
(the reference's +eps terms are negligible: ~1e-7 relative)

Sharding: pure data-parallel over B — core c owns batches [4c, 4c+4).
No collectives. Host pre-swizzles inputs so every DMA is a fully
contiguous stream:
    kt (4, 64, 4096): keys^T per batch (d on partitions 0:64) so score
        matmuls contract over d with K=64 at base partition 0 (untiled PE
        mode -- row-tiled tile_position mode-switching crashes on HW).
    vx (4, 128, 32, 65): vx[b, p, t, :64] = values[b, 128t+p, :], [.,64]=1
        -> mm2 rhs tiles with an appended ones column (denominator).
    qt (64, 32): qt[d, 8b+m] = query[b, m, d]
"""

import sys

sys.path.insert(0, "/opt/trn_rl_repo")

import numpy as np

import concourse.bass as bass
import concourse.bacc as bacc
import concourse.tile as tile
from concourse import mybir
from concourse.bass_utils import run_bass_kernel_spmd

N_CORES = 8
B, N, NQ, D, DV = 32, 4096, 8, 64, 64
BPC = B // N_CORES  # batches per core
NG = 4  # kt column groups per batch
NX = 512  # n elements per (j, group)
NT = 32  # 128-row n-subtiles per batch
FP = mybir.dt.float32

TRACE = False  # test.py flips this to get exec_time_ns
LAST_RESULT = {}


def _ensure_ntff_hook():
    """The agent image's `antenv` lacks the `axon_hooks` submodule that
    bass_utils' trace path imports. Recreate it and register the ctypes
    NTFF profiling hook from trn_boot."""
    import types

    import antenv

    if hasattr(antenv, "axon_hooks"):
        return
    mod = types.ModuleType("antenv.axon_hooks")
    state = {"hook": None}
    mod.set_axon_ntff_profile_hook = lambda h: state.update(hook=h)
    mod.get_axon_ntff_profile_hook = lambda: state["hook"]
    sys.modules["antenv.axon_hooks"] = mod
    antenv.axon_hooks = mod
    try:
        sys.path.insert(0, "/root/.axon_site")
        from trn_agent_boot.trn_boot import _ntff_profile_via_ctypes

        mod.set_axon_ntff_profile_hook(
            _ntff_profile_via_ctypes("/opt/axon/libaxon_pjrt.so")
        )
    except Exception as exc:  # degrade to no tracing
        print(f"ntff hook unavailable: {exc}", file=sys.stderr)


def _build_graph() -> bass.Bass:
    # walrus' LDWEIGHTS lowering accepts only ONE sync wait per matmul, so
    # every PE instruction that joins multiple producers must have all but
    # one dependency already observed by an earlier PE instruction:
    #  - a throwaway matmul reading qt absorbs the qt-DMA wait before the
    #    first scores matmul (its output region is overwritten right after);
    #  - each batch's accumulation group opens with a denominator matmul
    #    whose rhs is a memset ones column (DVE dep only, merged with the
    #    p-tile DVE dep), so the first vx-reading matmul carries just the
    #    vx-DMA wait.
    # Residual multi-wait spots (tail drain etc.) are split legally by
    # Bacc.compile()'s generate_event_semaphores pass.
    nc = bacc.Bacc()
    kt = nc.declare_dram_parameter("kt", [BPC, D, N], FP, isOutput=False)
    vx = nc.declare_dram_parameter("vx", [BPC, 128, NT, DV + 1], FP, isOutput=False)
    qt = nc.declare_dram_parameter("qt", [D, BPC * NQ], FP, isOutput=False)
    out = nc.declare_dram_parameter("out", [BPC, NQ, DV], FP, isOutput=True)

    with tile.TileContext(nc) as tc:
        with (
            tc.tile_pool(name="consts", bufs=1) as consts,
            tc.tile_pool(name="kpool", bufs=BPC) as kpool,
            tc.tile_pool(name="vpool", bufs=BPC) as vpool,
            tc.tile_pool(name="epool", bufs=BPC) as epool,
            tc.tile_pool(name="ppool", bufs=BPC) as ppool,
            tc.tile_pool(name="stats", bufs=2 * BPC) as stats,
            tc.tile_pool(name="res", bufs=BPC) as respool,
            tc.tile_pool(name="spsum", bufs=BPC, space="PSUM") as spsum,
            tc.tile_pool(name="opsum", bufs=BPC, space="PSUM") as opsum,
        ):
            qt_s = consts.tile([D, BPC * NQ], FP)
            nc.sync.dma_start(out=qt_s, in_=qt[:])
            ones = consts.tile([128, 1], FP)
            nc.vector.memset(ones, 1.0)

            kt_tiles, vx_tiles, sc_tiles = [], [], []
            for b in range(BPC):
                # vx before kt: scores(b)'s wait on kt(b) then transitively
                # covers vx(b), so mm2's first matmul needs only the DVE wait.
                vx_s = vpool.tile([128, NT, DV + 1], FP)
                nc.sync.dma_start(out=vx_s, in_=vx[b])
                kt_s = kpool.tile([D, N], FP)
                nc.sync.dma_start(out=kt_s, in_=kt[b])
                kt_tiles.append(kt_s)
                vx_tiles.append(vx_s)

            def scores(b):
                sc = spsum.tile([128, NT, NQ], FP)
                sc_tiles.append(sc)
                kt_s = kt_tiles[b]
                for t in range(NT):
                    nc.tensor.matmul(
                        out=sc[:, t, :],
                        lhsT=kt_s[:, 128 * t : 128 * (t + 1)],
                        rhs=qt_s[:, NQ * b : NQ * (b + 1)],
                        start=True,
                        stop=True,
                    )

            scores(0)
            for b in range(BPC):
                if b + 1 < BPC:
                    scores(b + 1)

                # softmax over the 8 slots (free dim), batch-wide ops
                e_s = epool.tile([128, NT, NQ], FP)
                nc.scalar.activation(
                    out=e_s,
                    in_=sc_tiles[b][:],
                    func=mybir.ActivationFunctionType.Exp,
                    scale=0.125,  # 1/sqrt(64)
                )
                rs = stats.tile([128, NT], FP, tag="rs")
                nc.vector.reduce_sum(out=rs, in_=e_s, axis=mybir.AxisListType.X)
                rr = stats.tile([128, NT], FP, tag="rr")
                nc.vector.reciprocal(out=rr, in_=rs)
                p_s = ppool.tile([128, NT, NQ], FP)
                rr_ap = rr[:]
                rr_bcast = bass.AP(
                    tensor=rr_ap.tensor,
                    offset=rr_ap.offset,
                    ap=[rr_ap.ap[0], rr_ap.ap[1], [0, NQ]],
                )
                nc.vector.tensor_mul(out=p_s, in0=e_s, in1=rr_bcast)

                # out_acc[m, v] = sum_n p[n, m] * vx[n, v]   (col 64 = den)
                out_ps = opsum.tile([NQ, DV + 1], FP)
                for t in range(NT):
                    nc.tensor.matmul(
                        out=out_ps,
                        lhsT=p_s[:, t, :],
                        rhs=vx_tiles[b][:, t, :],
                        start=(t == 0),
                        stop=(t == NT - 1),
                    )

                rden = stats.tile([NQ, 1], FP, tag="rden")
                nc.vector.reciprocal(out=rden, in_=out_ps[:, DV : DV + 1])
                res = respool.tile([NQ, DV], FP)
                nc.vector.tensor_scalar_mul(out=res, in0=out_ps[:, 0:DV], scalar1=rden)
                # SWDGE lane: fresh per-batch lane, so only the DVE wait
                # (HWDGE lanes are all taken by input DMAs -> chain wait
                # would exceed walrus' 1-wait DMA limit)
                nc.gpsimd.dma_start(out=out[b], in_=res)

    nc.compile()
    return nc


_NC = None


def _shard_inputs(keys, values, query):
    keys = np.ascontiguousarray(keys, dtype=np.float32)
    values = np.ascontiguousarray(values, dtype=np.float32)
    query = np.ascontiguousarray(query, dtype=np.float32)
    in_maps = []
    for c in range(N_CORES):
        ks = keys[BPC * c : BPC * (c + 1)]
        kt = np.ascontiguousarray(ks.transpose(0, 2, 1))  # (BPC, D, N)

        vs = values[BPC * c : BPC * (c + 1)].reshape(BPC, NT, 128, DV)
        vx = np.empty((BPC, 128, NT, DV + 1), np.float32)
        vx[..., :DV] = vs.transpose(0, 2, 1, 3)
        vx[..., DV] = 1.0

        q = query[BPC * c : BPC * (c + 1)]  # (BPC, 8, 64)
        qt = np.ascontiguousarray(q.transpose(2, 0, 1).reshape(D, BPC * NQ))

        in_maps.append({"kt": kt, "vx": np.ascontiguousarray(vx), "qt": qt})
    return in_maps


def kernel(keys, values, query):
    global _NC
    if _NC is None:
        _NC = _build_graph()
    in_maps = _shard_inputs(keys, values, query)
    if TRACE:
        _ensure_ntff_hook()
    r = run_bass_kernel_spmd(_NC, in_maps, core_ids=list(range(N_CORES)), trace=TRACE)
    LAST_RESULT["exec_time_ns"] = r.exec_time_ns
    LAST_RESULT["results"] = r
    return np.concatenate([r.results[c]["out"] for c in range(N_CORES)], axis=0)


# revision 19
# speedup vs baseline: 2.7469x; 2.7469x over previous
"""Slot-attention kernel for Trainium2, SPMD over 8 NeuronCores.

Math (per batch b):
    s = keys @ query.T / sqrt(64)            # (N, 8)
    e = exp(s)                               # softmax over slots, no max-sub
    p = e / rowsum(e)                        # (N, 8)
    out = (p.T @ values) / (p.T @ ones)      # (8, 64)
(the reference's +eps terms are negligible: ~1e-7 relative)

Sharding: pure data-parallel over B -- core c owns batches [4c, 4c+4).
No collectives. The host pre-swizzles inputs (bf16) so every DMA is a
fully contiguous 128-partition stream and every matmul is untiled K=128:

  kt (BPC, 128, 16, 128) bf16:
      kt[b, 64j+d, u, i] = keys[b, 128*(2u+j)+i, d]
      Two consecutive 128-row n-tiles (j=0,1) stacked on the partition
      axis form one K=128 stationary operand per pair u.
  qz (128, BPC, 16) bf16: block-diagonal query so one matmul yields both
      tiles' scores: qz[0:64, b, 0:8] = qz[64:128, b, 8:16] = query[b].T
      (zeros elsewhere kill the cross terms).
  vx (BPC, 128, 32, 65) bf16: vx[b, p, t, :64] = values[b, 128t+p, :],
      [..., 64] = 1 -> the ones column accumulates the denominator.

fp32 matmuls on TRN2 run twice (fp32_mode=LOW/HIGH passes); bf16 runs
once and enables Fast Weight Load. Accumulation stays fp32 in PSUM and
the softmax/epilogue arithmetic is fp32, so only input rounding is bf16
(~1e-3 rel err, gate is 2e-2).
"""

import sys

sys.path.insert(0, "/opt/trn_rl_repo")

import numpy as np

import concourse.bass as bass
import concourse.bacc as bacc
import concourse.tile as tile
from concourse import mybir
from concourse.bass_utils import run_bass_kernel_spmd

N_CORES = 8
B, N, NQ, D, DV = 32, 4096, 8, 64, 64
BPC = B // N_CORES  # batches per core
NT = 32  # 128-row n-subtiles per batch
NU = NT // 2  # stacked pairs per batch
FP = mybir.dt.float32
BF = mybir.dt.bfloat16

TRACE = False  # test.py flips this to get exec_time_ns
LAST_RESULT = {}


def _ensure_ntff_hook():
    """The agent image's `antenv` lacks the `axon_hooks` submodule that
    bass_utils' trace path imports. Recreate it and register the ctypes
    NTFF profiling hook from trn_boot."""
    import types

    import antenv

    if hasattr(antenv, "axon_hooks"):
        return
    mod = types.ModuleType("antenv.axon_hooks")
    state = {"hook": None}
    mod.set_axon_ntff_profile_hook = lambda h: state.update(hook=h)
    mod.get_axon_ntff_profile_hook = lambda: state["hook"]
    sys.modules["antenv.axon_hooks"] = mod
    antenv.axon_hooks = mod
    try:
        sys.path.insert(0, "/root/.axon_site")
        from trn_agent_boot.trn_boot import _ntff_profile_via_ctypes

        mod.set_axon_ntff_profile_hook(
            _ntff_profile_via_ctypes("/opt/axon/libaxon_pjrt.so")
        )
    except Exception as exc:  # degrade to no tracing
        print(f"ntff hook unavailable: {exc}", file=sys.stderr)


def _build_graph() -> bass.Bass:
    nc = bacc.Bacc()
    kt = nc.declare_dram_parameter("kt", [BPC, 128, NU, 128], BF, isOutput=False)
    vx = nc.declare_dram_parameter("vx", [BPC, 128, NT, DV + 1], BF, isOutput=False)
    qz = nc.declare_dram_parameter("qz", [128, BPC * 16], BF, isOutput=False)
    out = nc.declare_dram_parameter("out", [BPC, NQ, DV], FP, isOutput=True)

    with tile.TileContext(nc) as tc:
        with (
            tc.tile_pool(name="consts", bufs=1) as consts,
            tc.tile_pool(name="kpool", bufs=BPC) as kpool,
            tc.tile_pool(name="vpool", bufs=BPC) as vpool,
            tc.tile_pool(name="epool", bufs=BPC) as epool,
            tc.tile_pool(name="ppool", bufs=BPC) as ppool,
            tc.tile_pool(name="stats", bufs=2 * BPC) as stats,
            tc.tile_pool(name="res", bufs=BPC) as respool,
            tc.tile_pool(name="spsum", bufs=BPC, space="PSUM") as spsum,
            tc.tile_pool(name="opsum", bufs=BPC, space="PSUM") as opsum,
        ):
            qz_s = consts.tile([128, BPC * 16], BF)
            nc.sync.dma_start(out=qz_s, in_=qz[:])

            kt_tiles, vx_tiles, sc_tiles = [], [], []
            for b in range(BPC):
                # vx on the ACT HWDGE ring, kt on the SP ring -> the two
                # input streams run on different hardware DMA queues.
                vx_s = vpool.tile([128, NT, DV + 1], BF)
                nc.scalar.dma_start(out=vx_s, in_=vx[b])
                kt_s = kpool.tile([128, NU, 128], BF)
                nc.sync.dma_start(out=kt_s, in_=kt[b])
                kt_tiles.append(kt_s)
                vx_tiles.append(vx_s)

            def scores(b):
                sc = spsum.tile([128, NT, NQ], FP)
                sc_tiles.append(sc)
                kt_s = kt_tiles[b]
                for u in range(NU):
                    # one K=128 matmul -> scores of tiles 2u and 2u+1
                    nc.tensor.matmul(
                        out=sc[:].rearrange("p t m -> p (t m)")[:, 16 * u : 16 * (u + 1)],
                        lhsT=kt_s[:, u, :],
                        rhs=qz_s[:, 16 * b : 16 * (b + 1)],
                        start=True,
                        stop=True,
                    )

            scores(0)
            for b in range(BPC):
                if b + 1 < BPC:
                    scores(b + 1)

                # softmax over the 8 slots (free dim), batch-wide fp32 ops
                e_s = epool.tile([128, NT, NQ], FP)
                nc.scalar.activation(
                    out=e_s,
                    in_=sc_tiles[b][:],
                    func=mybir.ActivationFunctionType.Exp,
                    scale=0.125,  # 1/sqrt(64)
                )
                rs = stats.tile([128, NT], FP, tag="rs")
                nc.vector.reduce_sum(out=rs, in_=e_s, axis=mybir.AxisListType.X)
                rr = stats.tile([128, NT], FP, tag="rr")
                nc.vector.reciprocal(out=rr, in_=rs)
                p_s = ppool.tile([128, NT, NQ], BF)
                rr_ap = rr[:]
                rr_bcast = bass.AP(
                    tensor=rr_ap.tensor,
                    offset=rr_ap.offset,
                    ap=[rr_ap.ap[0], rr_ap.ap[1], [0, NQ]],
                )
                nc.vector.tensor_mul(out=p_s, in0=e_s, in1=rr_bcast)

                # out_acc[m, v] = sum_n p[n, m] * vx[n, v]   (col 64 = den)
                out_ps = opsum.tile([NQ, DV + 1], FP)
                for t in range(NT):
                    nc.tensor.matmul(
                        out=out_ps,
                        lhsT=p_s[:, t, :],
                        rhs=vx_tiles[b][:, t, :],
                        start=(t == 0),
                        stop=(t == NT - 1),
                    )

                rden = stats.tile([NQ, 1], FP, tag="rden")
                nc.vector.reciprocal(out=rden, in_=out_ps[:, DV : DV + 1])
                res = respool.tile([NQ, DV], FP)
                nc.vector.tensor_scalar_mul(out=res, in0=out_ps[:, 0:DV], scalar1=rden)
                # SWDGE lane: fresh per-batch lane -> only the DVE wait
                nc.gpsimd.dma_start(out=out[b], in_=res)

    nc.compile()
    return nc


_NC = None


def _shard_inputs(keys, values, query):
    import ml_dtypes

    bf16 = ml_dtypes.bfloat16
    keys = np.ascontiguousarray(keys, dtype=np.float32)
    values = np.ascontiguousarray(values, dtype=np.float32)
    query = np.ascontiguousarray(query, dtype=np.float32)
    in_maps = []
    for c in range(N_CORES):
        ks = keys[BPC * c : BPC * (c + 1)]  # (BPC, N, D)
        # kt[b, 64j+d, u, i] = keys[b, 128*(2u+j)+i, d]
        kt = ks.reshape(BPC, NU, 2, 128, D).transpose(0, 2, 4, 1, 3)
        kt = np.ascontiguousarray(kt.reshape(BPC, 128, NU, 128), dtype=bf16)

        vs = values[BPC * c : BPC * (c + 1)].reshape(BPC, NT, 128, DV)
        vx = np.empty((BPC, 128, NT, DV + 1), bf16)
        vx[..., :DV] = vs.transpose(0, 2, 1, 3).astype(bf16)
        vx[..., DV] = 1.0

        q = query[BPC * c : BPC * (c + 1)]  # (BPC, 8, 64)
        qz = np.zeros((128, BPC, 16), np.float32)
        qz[0:64, :, 0:NQ] = q.transpose(2, 0, 1)
        qz[64:128, :, NQ : 2 * NQ] = q.transpose(2, 0, 1)
        qz = np.ascontiguousarray(qz.reshape(128, BPC * 16), dtype=bf16)

        in_maps.append({"kt": kt, "vx": vx, "qz": qz})
    return in_maps


def kernel(keys, values, query):
    global _NC
    if _NC is None:
        _NC = _build_graph()
    in_maps = _shard_inputs(keys, values, query)
    if TRACE:
        _ensure_ntff_hook()
    r = run_bass_kernel_spmd(_NC, in_maps, core_ids=list(range(N_CORES)), trace=TRACE)
    LAST_RESULT["exec_time_ns"] = r.exec_time_ns
    LAST_RESULT["results"] = r
    return np.concatenate([r.results[c]["out"] for c in range(N_CORES)], axis=0)


# revision 25
# speedup vs baseline: 3.1375x; 1.1422x over previous
"""Slot-attention kernel for Trainium2, SPMD over 8 NeuronCores (raw bacc).

Math (per batch b):
    s = keys @ query.T / sqrt(64)            # (N, 8)
    p = exp(s) / rowsum(exp(s))              # softmax over 8 slots
    out = (p.T @ values) / (p.T @ ones)      # (8, 64)
(the reference's +eps terms are negligible: ~1e-7 relative)

Sharding: pure data-parallel over B -- core c owns batches [4c, 4c+4).
No collectives. Host pre-swizzles inputs to bf16 so every DMA is a fully
contiguous 128-partition stream and every matmul is untiled K=128:

  kt (BPC, 128, 16, 128): kt[b, 64j+d, u, i] = keys[b, 128*(2u+j)+i, d]
     two consecutive 128-row n-tiles stacked on partitions = K=128 lhsT.
  qz (128, BPC*16): block-diagonal query replicas; one 16-col matmul per
     pair u yields both tiles' scores (zeros kill cross terms).
  vx (BPC, 128, 32, 65): values rows on partitions + ones column for the
     denominator.

Raw bacc (no TileContext): hand-placed semaphores and PSUM banks avoid
Tile's ~12us of entry/exit barriers and per-semaphore reset storm.
Engine plan:
  SP   : qz + kt DMAs (HWDGE ring A)          -> SEM_KT  (+16 each)
  ACT  : vx DMAs (HWDGE ring B); exp per batch -> SEM_VX, SEM_E
  PE   : scores(b) [16 matmuls] -> SEM_SC; mm2(b) [32 matmuls] -> SEM_O
  DVE  : rowsum/recip/scale p(b) -> SEM_P; epilogue res(b) -> SEM_R
  POOL : out DMAs (SWDGE) -> SEM_OUT; final wait + sem_clear for re-exec
PSUM: sc(b) in bank b (PE-W then ACT-R, serialized by SEM_SC);
      o_ps(b) in bank 4+b (PE-W then DVE-R, serialized by SEM_O).
"""

import sys

sys.path.insert(0, "/opt/trn_rl_repo")

from contextlib import ExitStack

import numpy as np

import concourse.bacc as bacc
import concourse.bass as bass
from concourse import mybir
from concourse.bass_utils import run_bass_kernel_spmd

N_CORES = 8
B, N, NQ, D, DV = 32, 4096, 8, 64, 64
BPC = B // N_CORES  # batches per core
NT = 32  # 128-row n-subtiles per batch
NU = NT // 2  # stacked pairs per batch
FP = mybir.dt.float32
BF = mybir.dt.bfloat16

TRACE = False  # test.py flips this to get exec_time_ns
LAST_RESULT = {}


def _ensure_ntff_hook():
    """The agent image's `antenv` lacks the `axon_hooks` submodule that
    bass_utils' trace path imports. Recreate it and register the ctypes
    NTFF profiling hook from trn_boot."""
    import types

    import antenv

    if hasattr(antenv, "axon_hooks"):
        return
    mod = types.ModuleType("antenv.axon_hooks")
    state = {"hook": None}
    mod.set_axon_ntff_profile_hook = lambda h: state.update(hook=h)
    mod.get_axon_ntff_profile_hook = lambda: state["hook"]
    sys.modules["antenv.axon_hooks"] = mod
    antenv.axon_hooks = mod
    try:
        sys.path.insert(0, "/root/.axon_site")
        from trn_agent_boot.trn_boot import _ntff_profile_via_ctypes

        mod.set_axon_ntff_profile_hook(
            _ntff_profile_via_ctypes("/opt/axon/libaxon_pjrt.so")
        )
    except Exception as exc:  # degrade to no tracing
        print(f"ntff hook unavailable: {exc}", file=sys.stderr)


def _build_graph() -> bass.Bass:
    nc = bacc.Bacc()
    kt = nc.declare_dram_parameter("kt", [BPC, 128, NU, 128], BF, isOutput=False)
    vx = nc.declare_dram_parameter("vx", [BPC, 128, NT, DV + 1], BF, isOutput=False)
    qz = nc.declare_dram_parameter("qz", [128, BPC * 16], BF, isOutput=False)
    out = nc.declare_dram_parameter("out", [BPC, NQ, DV], FP, isOutput=True)

    ctx = ExitStack()
    with ctx:
        qz_s = ctx.enter_context(nc.sbuf_tensor("qz_s", [128, BPC * 16], BF))
        kt_s = [
            ctx.enter_context(nc.sbuf_tensor(f"kt_s{b}", [128, NU, 128], BF))
            for b in range(BPC)
        ]
        vx_s = [
            ctx.enter_context(nc.sbuf_tensor(f"vx_s{b}", [128, NT, DV + 1], BF))
            for b in range(BPC)
        ]
        e_s = [
            ctx.enter_context(nc.sbuf_tensor(f"e_s{b}", [128, NT, NQ], FP))
            for b in range(BPC)
        ]
        p_s = [
            ctx.enter_context(nc.sbuf_tensor(f"p_s{b}", [128, NT, NQ], BF))
            for b in range(BPC)
        ]
        rs_s = [
            ctx.enter_context(nc.sbuf_tensor(f"rs_s{b}", [128, NT], FP))
            for b in range(BPC)
        ]
        rr_s = [
            ctx.enter_context(nc.sbuf_tensor(f"rr_s{b}", [128, NT], FP))
            for b in range(BPC)
        ]
        rden_s = [
            ctx.enter_context(nc.sbuf_tensor(f"rden_s{b}", [NQ, 1], FP))
            for b in range(BPC)
        ]
        res_s = [
            ctx.enter_context(nc.sbuf_tensor(f"res_s{b}", [NQ, DV], FP))
            for b in range(BPC)
        ]
        # one full PSUM bank each: sc(b) -> bank b, o_ps(b) -> bank 4+b
        sc_ps = [
            ctx.enter_context(nc.psum_tensor(f"sc_ps{b}", [128, 512], FP))
            for b in range(BPC)
        ]
        o_ps = [
            ctx.enter_context(nc.psum_tensor(f"o_ps{b}", [128, 512], FP))
            for b in range(BPC)
        ]

        sems = {
            name: ctx.enter_context(nc.semaphore(name))
            for name in (
                ["QZ"]
                + [f"KT{b}" for b in range(BPC)]
                + [f"VX{b}" for b in range(BPC)]
                + ["SC", "E", "P", "O", "R", "OUT"]
            )
        }
        sem_lo = min(s.num for s in sems.values())
        sem_hi = max(s.num for s in sems.values())

        with nc.Block() as block:

            @block.sync
            def _(sp):
                sp.dma_start(out=qz_s[:], in_=qz[:]).then_inc(sems["QZ"], 16)
                for b in range(BPC):
                    sp.dma_start(out=kt_s[b][:], in_=kt[b]).then_inc(
                        sems[f"KT{b}"], 16
                    )

            @block.scalar
            def _(act):
                for b in range(BPC):
                    act.dma_start(out=vx_s[b][:], in_=vx[b]).then_inc(
                        sems[f"VX{b}"], 16
                    )
                for b in range(BPC):
                    act.wait_ge(sems["SC"], b + 1)
                    act.activation(
                        out=e_s[b][:],
                        in_=sc_ps[b][:].rearrange("p (t m) -> p t m", m=NQ)[
                            :, 0:NT, :
                        ],
                        func=mybir.ActivationFunctionType.Exp,
                        scale=0.125,  # 1/sqrt(64)
                    ).then_inc(sems["E"], 1)

            @block.tensor
            def _(pe):
                def scores(b):
                    if b == 0:
                        pe.wait_ge(sems["QZ"], 16)
                    pe.wait_ge(sems[f"KT{b}"], 16)
                    for u in range(NU):
                        mm = pe.matmul(
                            out=sc_ps[b][:, 16 * u : 16 * (u + 1)],
                            lhsT=kt_s[b][:, u, :],
                            rhs=qz_s[:, 16 * b : 16 * (b + 1)],
                            start=True,
                            stop=True,
                        )
                    mm.then_inc(sems["SC"], 1)

                def mm2(b):
                    pe.wait_ge(sems["P"], b + 1)
                    pe.wait_ge(sems[f"VX{b}"], 16)
                    for t in range(NT):
                        mm = pe.matmul(
                            out=o_ps[b][0:NQ, 0 : DV + 1],
                            lhsT=p_s[b][:, t, :],
                            rhs=vx_s[b][:, t, :],
                            start=(t == 0),
                            stop=(t == NT - 1),
                        )
                    mm.then_inc(sems["O"], 1)

                scores(0)
                for b in range(BPC):
                    if b + 1 < BPC:
                        scores(b + 1)
                    mm2(b)

            @block.vector
            def _(dve):
                for b in range(BPC):
                    dve.wait_ge(sems["E"], b + 1)
                    dve.reduce_sum(
                        out=rs_s[b][:], in_=e_s[b][:], axis=mybir.AxisListType.X
                    )
                    dve.drain()
                    dve.reciprocal(out=rr_s[b][:], in_=rs_s[b][:])
                    dve.drain()
                    rr_ap = rr_s[b][:]
                    rr_bcast = bass.AP(
                        tensor=rr_ap.tensor,
                        offset=rr_ap.offset,
                        ap=[rr_ap.ap[0], rr_ap.ap[1], [0, NQ]],
                    )
                    dve.tensor_mul(out=p_s[b][:], in0=e_s[b][:], in1=rr_bcast).then_inc(
                        sems["P"], 1
                    )
                for b in range(BPC):
                    dve.wait_ge(sems["O"], b + 1)
                    dve.reciprocal(out=rden_s[b][:], in_=o_ps[b][0:NQ, DV : DV + 1])
                    dve.drain()
                    dve.tensor_scalar_mul(
                        out=res_s[b][:],
                        in0=o_ps[b][0:NQ, 0:DV],
                        scalar1=rden_s[b][:],
                    ).then_inc(sems["R"], 1)

            @block.gpsimd
            def _(pool):
                for b in range(BPC):
                    pool.wait_ge(sems["R"], b + 1)
                    pool.dma_start(out=out[b], in_=res_s[b][:]).then_inc(
                        sems["OUT"], 16
                    )
                pool.wait_ge(sems["OUT"], 16 * BPC)

            # rendezvous all engines, then zero the kernel semaphores so a
            # second execution of the NEFF starts from clean state
            nc.all_engine_barrier()
            nc.gpsimd.sem_clear(range(sem_lo, sem_hi + 1))

        nc.compile()
    return nc


_NC = None


def _shard_inputs(keys, values, query):
    import ml_dtypes

    bf16 = ml_dtypes.bfloat16
    keys = np.ascontiguousarray(keys, dtype=np.float32)
    values = np.ascontiguousarray(values, dtype=np.float32)
    query = np.ascontiguousarray(query, dtype=np.float32)
    in_maps = []
    for c in range(N_CORES):
        ks = keys[BPC * c : BPC * (c + 1)]  # (BPC, N, D)
        # kt[b, 64j+d, u, i] = keys[b, 128*(2u+j)+i, d]
        kt = ks.reshape(BPC, NU, 2, 128, D).transpose(0, 2, 4, 1, 3)
        kt = np.ascontiguousarray(kt.reshape(BPC, 128, NU, 128), dtype=bf16)

        vs = values[BPC * c : BPC * (c + 1)].reshape(BPC, NT, 128, DV)
        vx = np.empty((BPC, 128, NT, DV + 1), bf16)
        vx[..., :DV] = vs.transpose(0, 2, 1, 3).astype(bf16)
        vx[..., DV] = 1.0

        q = query[BPC * c : BPC * (c + 1)]  # (BPC, 8, 64)
        qz = np.zeros((128, BPC, 16), np.float32)
        qz[0:64, :, 0:NQ] = q.transpose(2, 0, 1)
        qz[64:128, :, NQ : 2 * NQ] = q.transpose(2, 0, 1)
        qz = np.ascontiguousarray(qz.reshape(128, BPC * 16), dtype=bf16)

        in_maps.append({"kt": kt, "vx": vx, "qz": qz})
    return in_maps


def kernel(keys, values, query):
    global _NC
    if _NC is None:
        _NC = _build_graph()
    in_maps = _shard_inputs(keys, values, query)
    if TRACE:
        _ensure_ntff_hook()
    r = run_bass_kernel_spmd(_NC, in_maps, core_ids=list(range(N_CORES)), trace=TRACE)
    LAST_RESULT["exec_time_ns"] = r.exec_time_ns
    LAST_RESULT["results"] = r
    return np.concatenate([r.results[c]["out"] for c in range(N_CORES)], axis=0)
